# revision 7
# baseline (speedup 1.0000x reference)
"""Trainium2 Bass kernel: 5x5 reflect-padded box-filter mean (LocalMean).

Full input:  image (32, 3, 512, 512) f32
Full output: same shape; out[r,c] = mean of the 5x5 window of the
reflect-padded image.

Strategy (pure data parallel over 8 NeuronCores, 4 images per core):
- Host pre-pads H and W by 2 with reflect -> (4, 3, 516, 516) per core.
- On-chip the filter is separable:
  * vertical 5-tap sum via TensorE banded matmuls (two constant weight
    tiles: D [128,128] main band, E [4,128] tail band; weights 1/25),
  * horizontal 5-tap sum via 3 DVE adds + 1 Pool add over shifted
    slices of the PSUM intermediate.
- DMA (HBM in 12.8MB + out 12.6MB per core) is the roofline bottleneck.
"""

import numpy as np

N_CORES = 8
B, C, H, W = 32, 3, 512, 512
PB = B // N_CORES          # images per core
PAD = 2
HP, WP = H + 2 * PAD, W + 2 * PAD   # 516

_CACHE = {}


def _band_weights():
    # D[k, m] = 1/25 for 0 <= k-m <= 4 (vertical 5-tap window starting at
    # output row m of the padded block); E covers the 4 tail rows that
    # spill into the next 128-row block.
    k = np.arange(128)[:, None]
    m = np.arange(128)[None, :]
    d = ((k - m >= 0) & (k - m <= 4)).astype(np.float32) / 25.0
    i = np.arange(4)[:, None]
    e = ((128 + i - m >= 0) & (128 + i - m <= 4)).astype(np.float32) / 25.0
    return d, e


def _build(reps=1):
    import concourse.bacc as bacc
    import concourse.tile as tile
    from concourse import mybir

    f32 = mybir.dt.float32
    nc = bacc.Bacc("TRN2", target_bir_lowering=False, debug=False,
                   num_devices=N_CORES)
    x = nc.dram_tensor("x", [PB, C, HP, WP], f32, kind="ExternalInput").ap()
    wd = nc.dram_tensor("wd", [128, 128], f32, kind="ExternalInput").ap()
    we = nc.dram_tensor("we", [4, 128], f32, kind="ExternalInput").ap()
    y = nc.dram_tensor("y", [PB, C, H, W], f32, kind="ExternalOutput").ap()

    LOOKAHEAD = 2  # channel-images of input prefetched ahead of compute

    with tile.TileContext(nc) as tc:
        with (
            tc.tile_pool(name="wp", bufs=1) as wp,
            tc.tile_pool(name="xp", bufs=4 * (LOOKAHEAD + 2)) as xp,
            tc.tile_pool(name="xtp", bufs=LOOKAHEAD + 2) as xtp,
            tc.tile_pool(name="vp", bufs=3, space="PSUM") as vp,
            tc.tile_pool(name="vsp", bufs=4) as vsp,
            tc.tile_pool(name="tp", bufs=4) as tp,
            tc.tile_pool(name="op", bufs=6) as op,
        ):
            d_t = wp.tile([128, 128], f32)
            nc.sync.dma_start(d_t[:], wd[:, :])
            e_t = wp.tile([4, 128], f32)
            nc.sync.dma_start(e_t[:], we[:, :])

            cis = [(n, c) for _ in range(reps)
                   for n in range(PB) for c in range(C)]
            loaded = {}  # step index -> list of 5 X tiles

            def load(s):
                n, c = cis[s]
                xts = []
                for b in range(4):
                    t = xp.tile([128, WP], f32)
                    nc.sync.dma_start(t[:], x[n, c, 128 * b:128 * b + 128, :])
                    xts.append(t)
                t4 = xtp.tile([4, WP], f32)
                nc.sync.dma_start(t4[:], x[n, c, H:HP, :])
                xts.append(t4)
                loaded[s] = xts

            for s in range(min(LOOKAHEAD, len(cis))):
                load(s)

            blk = 0  # global block counter, for the DVE/Pool work split
            for s, (n, c) in enumerate(cis):
                if s + LOOKAHEAD < len(cis):
                    load(s + LOOKAHEAD)
                xts = loaded.pop(s)

                for b in range(4):
                    v = vp.tile([128, WP], f32)
                    tail = xts[b + 1]
                    # V = D.T @ X_b + E.T @ X_tail, split at the PSUM
                    # bank boundary (fp32 matmul N <= 512).
                    nc.tensor.matmul(v[:, 0:512], d_t[:], xts[b][:, 0:512],
                                     start=True, stop=False)
                    nc.tensor.matmul(v[:, 512:516], d_t[:], xts[b][:, 512:516],
                                     start=True, stop=False)
                    nc.tensor.matmul(v[:, 0:512], e_t[:], tail[0:4, 0:512],
                                     start=False, stop=True)
                    nc.tensor.matmul(v[:, 512:516], e_t[:], tail[0:4, 512:516],
                                     start=False, stop=True)

                    # PSUM -> SBUF once (DVE/Pool ops may read at most one
                    # PSUM operand, Pool none), on the idle ScalarE.
                    vs = vsp.tile([128, WP], f32)
                    nc.scalar.copy(vs[:], v[:])
                    # Horizontal 5-tap: out = sum_{d=0..4} Vs[:, d:d+512].
                    # t1 on DVE and t2 on Pool run concurrently; t3 rotates
                    # 1-in-3 onto Pool to balance DVE vs Pool load.
                    t1 = tp.tile([128, W], f32)
                    nc.vector.tensor_add(t1[:], vs[:, 0:512], vs[:, 1:513])
                    t2 = tp.tile([128, W], f32)
                    nc.gpsimd.tensor_add(t2[:], vs[:, 2:514], vs[:, 3:515])
                    t3 = tp.tile([128, W], f32)
                    eng = nc.gpsimd if blk % 3 == 2 else nc.vector
                    eng.tensor_add(t3[:], t1[:], t2[:])
                    o = op.tile([128, W], f32)
                    nc.vector.tensor_add(o[:], t3[:], vs[:, 4:516])
                    nc.sync.dma_start(y[n, c, 128 * b:128 * b + 128, :], o[:])
                    blk += 1

    nc.compile()
    return nc


def _get_nc(reps=1):
    key = ("nc", reps)
    if key not in _CACHE:
        _CACHE[key] = _build(reps)
    return _CACHE[key]


def _shard_inputs(image: np.ndarray):
    image = np.ascontiguousarray(np.asarray(image, dtype=np.float32))
    padded = np.pad(image, ((0, 0), (0, 0), (PAD, PAD), (PAD, PAD)),
                    mode="reflect")
    d, e = _band_weights()
    in_maps = []
    for i in range(N_CORES):
        in_maps.append({
            "x": np.ascontiguousarray(padded[i * PB:(i + 1) * PB]),
            "wd": d,
            "we": e,
        })
    return in_maps


def kernel(image: np.ndarray) -> np.ndarray:
    from concourse import bass_utils

    nc = _get_nc()
    in_maps = _shard_inputs(image)
    res = bass_utils.run_bass_kernel_spmd(nc, in_maps,
                                          core_ids=list(range(N_CORES)))
    return np.concatenate([res.results[i]["y"] for i in range(N_CORES)], axis=0)


# revision 11
# speedup vs baseline: 1.3073x; 1.3073x over previous
"""Trainium2 Bass kernel: 5x5 reflect-padded box-filter mean (LocalMean).

Full input:  image (32, 3, 512, 512) f32
Full output: same shape; out[r,c] = mean of the 5x5 window of the
reflect-padded image.

Strategy (pure data parallel over 8 NeuronCores, 4 images per core):
- Host pre-pads H and W by 2 with reflect -> (4, 3, 516, 516) per core.
- On-chip the filter is separable:
  * vertical 5-tap sum via TensorE banded matmuls (constant lower-band
    weight tile, 1/25-scaled; row blocks of 124 output rows so each
    block's 128 input rows live in a single SBUF tile -> one matmul),
  * horizontal 5-tap sum via one DVE reduce (window head) plus one DVE
    tensor_tensor_scan per block: H[j] = (V[j+4] + H[j-1]) - V[j-1].
- ScalarE copies the PSUM intermediate to SBUF (scan operands may not
  both live in PSUM); DMA (HBM ~13MB in + 12.6MB out per core) is the
  roofline bottleneck. GPSIMD is intentionally unused (2-input
  elementwise there is several times slower than DVE and contends for
  the DVE SBUF port).
"""

import numpy as np

N_CORES = 8
B, C, H, W = 32, 3, 512, 512
PB = B // N_CORES          # images per core
PAD = 2
HP, WP = H + 2 * PAD, W + 2 * PAD   # 516

# Output-row blocks of 124 (last 16): input rows [124b, 124b+128) per
# block all sit in one 128-partition tile, so the vertical matmul needs
# no cross-tile tail accumulation.
BLOCKS = [(0, 124), (124, 124), (248, 124), (372, 124), (496, 16)]

_CACHE = {}
# Experiment switches (default = the shipped configuration).
_CFG = {}


def _band_weights():
    # W[k, m] = 1/25 for 0 <= k-m <= 4: vertical 5-tap window starting at
    # output row m reads input rows m..m+4 of the padded block.
    def band(K, M):
        k = np.arange(K)[:, None]
        m = np.arange(M)[None, :]
        return (((k - m) >= 0) & ((k - m) <= 4)).astype(np.float32) / 25.0
    return band(128, 124), band(20, 16)


def _build(reps=1):
    import concourse.bacc as bacc
    import concourse.tile as tile
    from concourse import mybir

    f32 = mybir.dt.float32
    nc = bacc.Bacc("TRN2", target_bir_lowering=False, debug=False,
                   num_devices=N_CORES)
    x = nc.dram_tensor("x", [PB, C, HP, WP], f32, kind="ExternalInput").ap()
    wd = nc.dram_tensor("wd", [128, 124], f32, kind="ExternalInput").ap()
    wl = nc.dram_tensor("wl", [20, 16], f32, kind="ExternalInput").ap()
    y = nc.dram_tensor("y", [PB, C, H, W], f32, kind="ExternalOutput").ap()

    LOOKAHEAD = 2  # channel-images of input prefetched ahead of compute

    with tile.TileContext(nc) as tc:
        with (
            tc.tile_pool(name="wp", bufs=1) as wp,
            tc.tile_pool(name="xp", bufs=4 * (LOOKAHEAD + 2)) as xp,
            tc.tile_pool(name="xtp", bufs=LOOKAHEAD + 2) as xtp,
            tc.tile_pool(name="vp", bufs=3, space="PSUM") as vp,
            tc.tile_pool(name="vsp", bufs=4) as vsp,
            tc.tile_pool(name="op", bufs=6) as op,
        ):
            d_t = wp.tile([128, 124], f32)
            nc.sync.dma_start(d_t[:], wd[:, :])
            l_t = wp.tile([20, 16], f32)
            nc.sync.dma_start(l_t[:], wl[:, :])

            cis = [(n, c) for _ in range(reps)
                   for n in range(PB) for c in range(C)]
            loaded = {}  # step index -> list of 5 X tiles

            def load(s):
                n, c = cis[s]
                xts = []
                for b, (r0, h) in enumerate(BLOCKS):
                    kh = 128 if h == 124 else 20
                    pool = xp if kh == 128 else xtp
                    t = pool.tile([kh, WP], f32)
                    nc.sync.dma_start(t[:], x[n, c, r0:r0 + kh, :])
                    xts.append(t)
                loaded[s] = xts

            for s in range(min(LOOKAHEAD, len(cis))):
                load(s)

            for s, (n, c) in enumerate(cis):
                if s + LOOKAHEAD < len(cis):
                    load(s + LOOKAHEAD)
                xts = loaded.pop(s)

                for b, (r0, h) in enumerate(BLOCKS):
                    w_t = d_t if h == 124 else l_t
                    xt = xts[b]
                    v = vp.tile([128, WP], f32)
                    # V[m, :] = sum_{d=0..4} X[m+d, :] / 25, via banded
                    # matmul; N split at the PSUM bank boundary (fp32
                    # matmul N <= 512).
                    nc.tensor.matmul(v[0:h, 0:512], w_t[:], xt[:, 0:512],
                                     start=True, stop=True)
                    nc.tensor.matmul(v[0:h, 512:516], w_t[:], xt[:, 512:516],
                                     start=True, stop=True)

                    # PSUM -> SBUF once on the otherwise-idle ScalarE (the
                    # scan may read at most one PSUM operand; this also
                    # keeps DVE streams at SBUF rates).
                    vs = vsp.tile([128, WP], f32)
                    nc.scalar.copy(vs[0:h, :], v[0:h, :])

                    # Horizontal 5-tap sliding window on DVE:
                    #   H[0] = sum(Vs[0:5]);  H[j] = H[j-1] + Vs[j+4] - Vs[j-1]
                    o = op.tile([128, W], f32)
                    nc.vector.reduce_sum(o[0:h, 0:1], vs[0:h, 0:5],
                                         axis=mybir.AxisListType.X)
                    nc.vector.tensor_tensor_scan(
                        o[0:h, 1:512], vs[0:h, 5:516], vs[0:h, 0:511],
                        o[0:h, 0:1],
                        mybir.AluOpType.add, mybir.AluOpType.subtract)
                    nc.sync.dma_start(y[n, c, r0:r0 + h, :], o[0:h, :])

    nc.compile()
    return nc


def _get_nc(reps=1):
    key = ("nc", reps)
    if key not in _CACHE:
        _CACHE[key] = _build(reps)
    return _CACHE[key]


def _shard_inputs(image: np.ndarray):
    image = np.ascontiguousarray(np.asarray(image, dtype=np.float32))
    padded = np.pad(image, ((0, 0), (0, 0), (PAD, PAD), (PAD, PAD)),
                    mode="reflect")
    d, dl = _band_weights()
    in_maps = []
    for i in range(N_CORES):
        in_maps.append({
            "x": np.ascontiguousarray(padded[i * PB:(i + 1) * PB]),
            "wd": d,
            "wl": dl,
        })
    return in_maps


def kernel(image: np.ndarray) -> np.ndarray:
    from concourse import bass_utils

    nc = _get_nc()
    in_maps = _shard_inputs(image)
    res = bass_utils.run_bass_kernel_spmd(nc, in_maps,
                                          core_ids=list(range(N_CORES)))
    return np.concatenate([res.results[i]["y"] for i in range(N_CORES)], axis=0)


# revision 13
# speedup vs baseline: 1.3296x; 1.0170x over previous
"""Trainium2 Bass kernel: 5x5 reflect-padded box-filter mean (LocalMean).

Full input:  image (32, 3, 512, 512) f32
Full output: same shape; out[r,c] = mean of the 5x5 window of the
reflect-padded image.

Strategy (pure data parallel over 8 NeuronCores, 4 images per core):
- Host pre-pads H and W by 2 with reflect -> (4, 3, 516, 516) per core.
- On-chip the filter is separable:
  * vertical 5-tap sum via TensorE banded matmuls (constant lower-band
    weight tile, 1/25-scaled; row blocks of 124 output rows so each
    block's 128 input rows live in a single SBUF tile -> one matmul),
  * horizontal 5-tap sum via one DVE reduce (window head) plus one DVE
    tensor_tensor_scan per block: H[j] = (V[j+4] + H[j-1]) - V[j-1].
- ScalarE copies the PSUM intermediate to SBUF (scan operands may not
  both live in PSUM); DMA (HBM ~13MB in + 12.6MB out per core) is the
  roofline bottleneck. GPSIMD is intentionally unused (2-input
  elementwise there is several times slower than DVE and contends for
  the DVE SBUF port).
"""

import numpy as np

N_CORES = 8
B, C, H, W = 32, 3, 512, 512
PB = B // N_CORES          # images per core
PAD = 2
HP, WP = H + 2 * PAD, W + 2 * PAD   # 516

# Output-row blocks of 124 (last 16): input rows [124b, 124b+128) per
# block all sit in one 128-partition tile, so the vertical matmul needs
# no cross-tile tail accumulation.
BLOCKS = [(0, 124), (124, 124), (248, 124), (372, 124), (496, 16)]

_CACHE = {}
# Experiment switches (default = the shipped configuration).
_CFG = {}


def _band_weights():
    # W[k, m] = 1/25 for 0 <= k-m <= 4: vertical 5-tap window starting at
    # output row m reads input rows m..m+4 of the padded block.
    def band(K, M):
        k = np.arange(K)[:, None]
        m = np.arange(M)[None, :]
        return (((k - m) >= 0) & ((k - m) <= 4)).astype(np.float32) / 25.0
    return band(128, 124), band(20, 16)


def _build(reps=1):
    import concourse.bacc as bacc
    import concourse.tile as tile
    from concourse import mybir

    f32 = mybir.dt.float32
    nc = bacc.Bacc("TRN2", target_bir_lowering=False, debug=False,
                   num_devices=N_CORES)
    x = nc.dram_tensor("x", [PB, C, HP, WP], f32, kind="ExternalInput").ap()
    wd = nc.dram_tensor("wd", [128, 124], f32, kind="ExternalInput").ap()
    wl = nc.dram_tensor("wl", [20, 16], f32, kind="ExternalInput").ap()
    y = nc.dram_tensor("y", [PB, C, H, W], f32, kind="ExternalOutput").ap()

    LOOKAHEAD = 2  # channel-images of input prefetched ahead of compute

    with tile.TileContext(nc) as tc:
        with (
            tc.tile_pool(name="wp", bufs=1) as wp,
            tc.tile_pool(name="xp", bufs=4 * (LOOKAHEAD + 2)) as xp,
            tc.tile_pool(name="xtp", bufs=LOOKAHEAD + 2) as xtp,
            tc.tile_pool(name="vp", bufs=4, space="PSUM") as vp,
            tc.tile_pool(name="vsp", bufs=4) as vsp,
            tc.tile_pool(name="op", bufs=6) as op,
        ):
            d_t = wp.tile([128, 124], f32)
            nc.sync.dma_start(d_t[:], wd[:, :])
            l_t = wp.tile([20, 16], f32)
            nc.sync.dma_start(l_t[:], wl[:, :])

            cis = [(n, c) for _ in range(reps)
                   for n in range(PB) for c in range(C)]
            loaded = {}  # step index -> list of 5 X tiles

            def load(s):
                n, c = cis[s]
                xts = []
                for b, (r0, h) in enumerate(BLOCKS):
                    kh = 128 if h == 124 else 20
                    pool = xp if kh == 128 else xtp
                    t = pool.tile([kh, WP], f32)
                    nc.sync.dma_start(t[:], x[n, c, r0:r0 + kh, :])
                    xts.append(t)
                loaded[s] = xts

            for s in range(min(LOOKAHEAD, len(cis))):
                load(s)

            for s, (n, c) in enumerate(cis):
                if s + LOOKAHEAD < len(cis):
                    load(s + LOOKAHEAD)
                xts = loaded.pop(s)

                for b, (r0, h) in enumerate(BLOCKS):
                    w_t = d_t if h == 124 else l_t
                    xt = xts[b]
                    v = vp.tile([128, WP], f32)
                    # V[m, :] = sum_{d=0..4} X[m+d, :] / 25, via banded
                    # matmul; N split at the PSUM bank boundary (fp32
                    # matmul N <= 512).
                    nc.tensor.matmul(v[0:h, 0:512], w_t[:], xt[:, 0:512],
                                     start=True, stop=True)
                    nc.tensor.matmul(v[0:h, 512:516], w_t[:], xt[:, 512:516],
                                     start=True, stop=True)

                    # PSUM -> SBUF once on the otherwise-idle ScalarE (the
                    # scan may read at most one PSUM operand; this also
                    # keeps DVE streams at SBUF rates).
                    vs = vsp.tile([128, WP], f32)
                    nc.scalar.copy(vs[0:h, :], v[0:h, :])

                    # Horizontal 5-tap sliding window on DVE:
                    #   H[0] = sum(Vs[0:5]);  H[j] = H[j-1] + Vs[j+4] - Vs[j-1]
                    o = op.tile([128, W], f32)
                    nc.vector.reduce_sum(o[0:h, 0:1], vs[0:h, 0:5],
                                         axis=mybir.AxisListType.X)
                    nc.vector.tensor_tensor_scan(
                        o[0:h, 1:512], vs[0:h, 5:516], vs[0:h, 0:511],
                        o[0:h, 0:1],
                        mybir.AluOpType.add, mybir.AluOpType.subtract)
                    # Output DMAs issue from the ACT HWDGE queue so the SP
                    # sequencer (which issues all input DMAs) never blocks
                    # on an output's semaphore wait; DMA *issue* cost
                    # (~0.65us each) on a single sequencer is otherwise
                    # itself at the roofline.
                    nc.scalar.dma_start(y[n, c, r0:r0 + h, :], o[0:h, :])

    nc.compile()
    return nc


def _get_nc(reps=1):
    key = ("nc", reps)
    if key not in _CACHE:
        _CACHE[key] = _build(reps)
    return _CACHE[key]


def _shard_inputs(image: np.ndarray):
    image = np.ascontiguousarray(np.asarray(image, dtype=np.float32))
    padded = np.pad(image, ((0, 0), (0, 0), (PAD, PAD), (PAD, PAD)),
                    mode="reflect")
    d, dl = _band_weights()
    in_maps = []
    for i in range(N_CORES):
        in_maps.append({
            "x": np.ascontiguousarray(padded[i * PB:(i + 1) * PB]),
            "wd": d,
            "wl": dl,
        })
    return in_maps


def kernel(image: np.ndarray) -> np.ndarray:
    from concourse import bass_utils

    nc = _get_nc()
    in_maps = _shard_inputs(image)
    res = bass_utils.run_bass_kernel_spmd(nc, in_maps,
                                          core_ids=list(range(N_CORES)))
    return np.concatenate([res.results[i]["y"] for i in range(N_CORES)], axis=0)


# revision 15
# speedup vs baseline: 1.3305x; 1.0007x over previous
"""Trainium2 Bass kernel: 5x5 reflect-padded box-filter mean (LocalMean).

Full input:  image (32, 3, 512, 512) f32
Full output: same shape; out[r,c] = mean of the 5x5 window of the
reflect-padded image.

Strategy (pure data parallel over 8 NeuronCores, 4 images per core):
- Host pre-pads H and W by 2 with reflect -> (4, 3, 516, 516) per core.
- On-chip the filter is separable:
  * vertical 5-tap sum via TensorE banded matmuls (constant lower-band
    weight tile, 1/25-scaled; row blocks of 124 output rows so each
    block's 128 input rows live in a single SBUF tile -> one matmul),
  * horizontal 5-tap sum via one DVE reduce (window head) plus one DVE
    tensor_tensor_scan per block: H[j] = (V[j+4] + H[j-1]) - V[j-1].
- ScalarE copies the PSUM intermediate to SBUF (scan operands may not
  both live in PSUM); DMA (HBM ~13MB in + 12.6MB out per core) is the
  roofline bottleneck. GPSIMD is intentionally unused (2-input
  elementwise there is several times slower than DVE and contends for
  the DVE SBUF port).
"""

import numpy as np

N_CORES = 8
B, C, H, W = 32, 3, 512, 512
PB = B // N_CORES          # images per core
PAD = 2
HP, WP = H + 2 * PAD, W + 2 * PAD   # 516

# Output-row blocks of 124 (last 16): input rows [124b, 124b+128) per
# block all sit in one 128-partition tile, so the vertical matmul needs
# no cross-tile tail accumulation.
BLOCKS = [(0, 124), (124, 124), (248, 124), (372, 124), (496, 16)]

_CACHE = {}
# Experiment switches (default = the shipped configuration).
_CFG = {}


def _band_weights():
    # W[k, m] = 1/25 for 0 <= k-m <= 4: vertical 5-tap window starting at
    # output row m reads input rows m..m+4 of the padded block.
    def band(K, M):
        k = np.arange(K)[:, None]
        m = np.arange(M)[None, :]
        return (((k - m) >= 0) & ((k - m) <= 4)).astype(np.float32) / 25.0
    return band(128, 124), band(20, 16)


def _build(reps=1):
    import concourse.bacc as bacc
    import concourse.tile as tile
    from concourse import mybir

    f32 = mybir.dt.float32
    nc = bacc.Bacc("TRN2", target_bir_lowering=False, debug=False,
                   num_devices=N_CORES)
    x = nc.dram_tensor("x", [PB, C, HP, WP], f32, kind="ExternalInput").ap()
    wd = nc.dram_tensor("wd", [128, 124], f32, kind="ExternalInput").ap()
    wl = nc.dram_tensor("wl", [20, 16], f32, kind="ExternalInput").ap()
    y = nc.dram_tensor("y", [PB, C, H, W], f32, kind="ExternalOutput").ap()

    LOOKAHEAD = 2  # channel-images of input prefetched ahead of compute

    with tile.TileContext(nc) as tc:
        with (
            tc.tile_pool(name="wp", bufs=1) as wp,
            tc.tile_pool(name="xp", bufs=4 * (LOOKAHEAD + 2)) as xp,
            tc.tile_pool(name="xtp", bufs=LOOKAHEAD + 2) as xtp,
            tc.tile_pool(name="vp", bufs=4, space="PSUM") as vp,
            tc.tile_pool(name="vsp", bufs=6) as vsp,
            tc.tile_pool(name="op", bufs=8) as op,
        ):
            d_t = wp.tile([128, 124], f32)
            nc.sync.dma_start(d_t[:], wd[:, :])
            l_t = wp.tile([20, 16], f32)
            nc.sync.dma_start(l_t[:], wl[:, :])

            cis = [(n, c) for _ in range(reps)
                   for n in range(PB) for c in range(C)]
            loaded = {}  # step index -> list of 5 X tiles

            def load(s):
                n, c = cis[s]
                xts = []
                for b, (r0, h) in enumerate(BLOCKS):
                    kh = 128 if h == 124 else 20
                    pool = xp if kh == 128 else xtp
                    t = pool.tile([kh, WP], f32)
                    nc.sync.dma_start(t[:], x[n, c, r0:r0 + kh, :])
                    xts.append(t)
                loaded[s] = xts

            for s in range(min(LOOKAHEAD, len(cis))):
                load(s)

            for s, (n, c) in enumerate(cis):
                if s + LOOKAHEAD < len(cis):
                    load(s + LOOKAHEAD)
                xts = loaded.pop(s)

                for b, (r0, h) in enumerate(BLOCKS):
                    w_t = d_t if h == 124 else l_t
                    xt = xts[b]
                    v = vp.tile([128, WP], f32)
                    # V[m, :] = sum_{d=0..4} X[m+d, :] / 25, via banded
                    # matmul; N split at the PSUM bank boundary (fp32
                    # matmul N <= 512).
                    nc.tensor.matmul(v[0:h, 0:512], w_t[:], xt[:, 0:512],
                                     start=True, stop=True)
                    nc.tensor.matmul(v[0:h, 512:516], w_t[:], xt[:, 512:516],
                                     start=True, stop=True)

                    # PSUM -> SBUF once on the otherwise-idle ScalarE (the
                    # scan may read at most one PSUM operand; this also
                    # keeps DVE streams at SBUF rates).
                    vs = vsp.tile([128, WP], f32)
                    nc.scalar.copy(vs[0:h, :], v[0:h, :])

                    # Horizontal 5-tap sliding window on DVE:
                    #   H[0] = sum(Vs[0:5]);  H[j] = H[j-1] + Vs[j+4] - Vs[j-1]
                    o = op.tile([128, W], f32)
                    nc.vector.reduce_sum(o[0:h, 0:1], vs[0:h, 0:5],
                                         axis=mybir.AxisListType.X)
                    nc.vector.tensor_tensor_scan(
                        o[0:h, 1:512], vs[0:h, 5:516], vs[0:h, 0:511],
                        o[0:h, 0:1],
                        mybir.AluOpType.add, mybir.AluOpType.subtract)
                    # Output DMAs alternate between the two HWDGE queues:
                    # DMA *issue* costs ~0.65us per dma_start on an in-order
                    # sequencer, so issue work must be spread — SP carries
                    # the input DMAs, ACT the PSUM->SBUF copies, and each
                    # takes half the output issues to balance at ~58us.
                    dma_eng = nc.scalar if (s * 5 + b) % 2 == 0 else nc.sync
                    dma_eng.dma_start(y[n, c, r0:r0 + h, :], o[0:h, :])

    nc.compile()
    return nc


def _get_nc(reps=1):
    key = ("nc", reps)
    if key not in _CACHE:
        _CACHE[key] = _build(reps)
    return _CACHE[key]


def _shard_inputs(image: np.ndarray):
    image = np.ascontiguousarray(np.asarray(image, dtype=np.float32))
    padded = np.pad(image, ((0, 0), (0, 0), (PAD, PAD), (PAD, PAD)),
                    mode="reflect")
    d, dl = _band_weights()
    in_maps = []
    for i in range(N_CORES):
        in_maps.append({
            "x": np.ascontiguousarray(padded[i * PB:(i + 1) * PB]),
            "wd": d,
            "wl": dl,
        })
    return in_maps


def kernel(image: np.ndarray) -> np.ndarray:
    from concourse import bass_utils

    nc = _get_nc()
    in_maps = _shard_inputs(image)
    res = bass_utils.run_bass_kernel_spmd(nc, in_maps,
                                          core_ids=list(range(N_CORES)))
    return np.concatenate([res.results[i]["y"] for i in range(N_CORES)], axis=0)
